# revision 18
# baseline (speedup 1.0000x reference)
"""Trainium2 Bass kernel for the channel-attention module.

Reference computation (B=16, N=4096, C=384, H=8, D=48):
    x_in = x @ conv_w.T + conv_b                      # 1x1 conv == linear
    q    = (x_in @ wq.T + bq)  -> [B,H,D,N]
    k, v = (x_in @ wkv.T + bkv) -> 2x [B,H,D,N]
    attn = softmax((q * N**-0.5) @ k^T, axis=-1)      # [B,H,D,D] (over N!)
    out  = attn @ v                                   # [B,H,D,N]
    out  = out.transpose(0,2,1,3).reshape(B,N,C)      # verbatim torch layout
    y    = out @ wp.T + bp

Strategy: pure data parallelism over B across 8 NeuronCores (2 batches per
core), no collectives.  The conv is folded into the q/k/v projections on the
host (w_eff = (w @ conv_w).T, b_eff = b + w @ conv_b).

q and k are never materialized: since the attention logits contract over N,
S_h = (X wq + 1 bq^T)_h^T (X wk + 1 bk^T)_h
    = wq_h^T G wk_h + u_h (x) bk_h + bq_h (x) (v_h + N bk_h)
with G = X^T X (Gram matrix, accumulated on-chip in PSUM), s = X^T 1 (folded
into G as an extra column via a ones-column appended to x on the host),
u = wq^T s, v = wk^T s.  The rank-1 bias terms are added with K=1 matmuls.

The awkward transpose(0,2,1,3).reshape is handled exactly with 128-element
flat blocks: flat index (di, h, n) -> block u = 256*di + 32*h + n//128 with
intra-block offset c' = n%128.  Stage 5 (attn @ v) produces AT[c', u] tiles
through a block-diagonal attn^T matrix; stage 6 reads columns u = 3*r + j
(stride-3 APs) as the K=128 slices of the final projection.

Perf structure (final, 199us -> 171us):
  - inputs/outputs host-tiled so every window DMA is contiguous per
    partition (~3KB descriptors, ~330 GB/s); output is bf16, untiled on
    the host.
  - bd zeros via on-chip memset (no HBM dependency before PE warmup).
  - issue order: b0 v/G | b1 v/G + b0 S-chain interleaved | b0 st5/6 + b1
    S-chain interleaved | b1 st5/6 -- the serial softmax chain hides under
    the other batch's dense matmul stream (PE executes in issue order).
  - stage 6 stays f32r x f32r: bf16 lhsT with f32r rhs is an illegal mix,
    and a bf16 `at` would hit the 2.5x-slower 16-bit strided-write path in
    the evacuation.  The at evacuation (strided writes forced by stage 6's
    stride-3 slicing) is 2 ops (DVE 240 + ACT 144 cols) because per-op
    fixed overhead is ~200-350ns.
  - attn^T via 4 paired PE transposes ([48,96] -> [96,48] stacks two
    heads); scatter into bd on the idle gpsimd SWDGE queue; x windows on
    the SP queue; weights on the ACT queue; y DMAs batched 4 chunks.
  - PSUM tag rings: v(+warmup):2 banks, g(G accum + at_ps + y_ps):5,
    chain(t/r/s/tr):1 = 8; at/y sharing g's ring is deadlock-safe because
    every slot-reuse wait points at an earlier-issued instruction.
  - Known-bad variants (measured): sparse stage-5 segments + multi-op evac
    (249us -- evac op overhead dominates); transposed stage 6 (illegal
    dtype mix); xnw on ACT / scatter on SP / ACT-heavy evac (+6us each).
    LDWEIGHTS is never elided (1 per matmul), so same-lhsT tricks (G
    symmetry, stage-5 column sparsity) lose to the weight-load tax.
"""

import sys
import types
from contextlib import ExitStack

import numpy as np

import concourse.bass as bass
import concourse.tile as tile
from concourse import bacc, mybir
from concourse.bass_utils import run_bass_kernel_spmd
from concourse.masks import make_identity

B, N, C, H, D = 16, 4096, 384, 8, 48
N_CORES = 8
BPC = B // N_CORES          # batches per core
NW = 512                    # token window for the v projection
NWIN = N // NW              # 8 windows
NCHUNK = N // 128           # 32 token chunks of 128
NG = NCHUNK // 4            # 8 output DMA groups
SCALE = float(N) ** -0.5    # 1/64
F32 = mybir.dt.float32
F32R = mybir.dt.float32r
BF16 = mybir.dt.bfloat16

def _install_ntff_hook():
    """The agent image's antenv lacks axon_hooks, so trn_boot's NTFF hook
    registration degrades silently and trace=True would crash.  Recreate the
    module and register the ctypes hook so profiling works."""
    try:
        import antenv

        if "antenv.axon_hooks" in sys.modules:
            return
        mod = types.ModuleType("antenv.axon_hooks")
        mod._hook = None
        mod.set_axon_ntff_profile_hook = lambda h: setattr(mod, "_hook", h)
        mod.get_axon_ntff_profile_hook = lambda: mod._hook
        sys.modules["antenv.axon_hooks"] = mod
        antenv.axon_hooks = mod
        from trn_agent_boot.trn_boot import _ntff_profile_via_ctypes

        mod.set_axon_ntff_profile_hook(
            _ntff_profile_via_ctypes("/opt/axon/libaxon_pjrt.so")
        )
    except Exception:
        pass


def build():
    nc = bacc.Bacc("TRN2", target_bir_lowering=False, debug=False,
                   num_devices=N_CORES)

    # Host-tiled inputs: every [128, ...] tile is contiguous per partition.
    xt_p = nc.declare_dram_parameter("xt", [BPC, NWIN, 128, 3, NW], BF16,
                                     isOutput=False)
    xn_p = nc.declare_dram_parameter("xn", [BPC, NWIN, 128, 4, C + 1], BF16,
                                     isOutput=False)
    wq_p = nc.declare_dram_parameter("wqT", [C, C], BF16, isOutput=False)
    wk_p = nc.declare_dram_parameter("wkT", [C, C], BF16, isOutput=False)
    wv_p = nc.declare_dram_parameter("wvT", [C, C], BF16, isOutput=False)
    wp_p = nc.declare_dram_parameter("wpT", [C, C], BF16, isOutput=False)
    bq_p = nc.declare_dram_parameter("bq", [1, C], F32R, isOutput=False)
    bk_p = nc.declare_dram_parameter("bk", [1, C], F32R, isOutput=False)
    bv_p = nc.declare_dram_parameter("bv", [C], F32, isOutput=False)
    bp_p = nc.declare_dram_parameter("bp", [C], F32, isOutput=False)
    outp = nc.declare_dram_parameter("out", [BPC, NG, 128, 4, C], BF16,
                                     isOutput=True)

    with tile.TileContext(nc) as tc, ExitStack() as ctx:
        const = ctx.enter_context(tc.tile_pool(name="const", bufs=1))
        xin = ctx.enter_context(tc.tile_pool(name="xin", bufs=6))
        xnp = ctx.enter_context(tc.tile_pool(name="xnp", bufs=6))
        big = ctx.enter_context(tc.tile_pool(name="big", bufs=2))
        big1 = ctx.enter_context(tc.tile_pool(name="big1", bufs=1))
        sm = ctx.enter_context(tc.tile_pool(name="sm", bufs=2))
        yout = ctx.enter_context(tc.tile_pool(name="yout", bufs=3))
        psum = ctx.enter_context(tc.tile_pool(name="ps", bufs=1, space="PSUM"))

        # ---- block-diag tiles (two sets, one per batch) zeroed on-chip ----
        bd = [[const.tile([128, C], BF16, tag=f"bd{s}_{i}",
                          name=f"bd{s}_{i}") for i in range(3)]
              for s in range(2)]
        for i in range(3):
            nc.vector.memset(bd[0][i][:], 0.0)
        for i in range(3):
            nc.gpsimd.memset(bd[1][i][:], 0.0)

        # PE warmup on the zeroed tiles (keeps HAM busy until x lands)
        warm_ps = psum.tile([128, NW], F32, tag="v", bufs=2, name="warm_ps")
        for i in range(8):
            nc.tensor.matmul(warm_ps[:, 0:256], bd[0][i % 3][:, 0:128],
                             bd[0][i % 3][:, 0:256],
                             start=(i == 0), stop=(i == 7))
        warm_sb = const.tile([128, 1], BF16)
        nc.vector.tensor_copy(warm_sb[:], warm_ps[:, 0:1])
        nc.scalar.dma_start(outp.ap()[0, 0, :, 0, 0:1], warm_sb[:])

        # ---- constants on the ACT DMA queue (SP queue streams x) ----------
        def load_w(param):
            t = const.tile([128, 3, C], param.dtype, tag=f"w_{param.name}")
            nc.scalar.dma_start(
                t[:], param.ap().rearrange("(kc p) o -> p kc o", p=128))
            return t

        wv_sb, wq_sb, wk_sb, wp_sb = (load_w(p) for p in
                                      (wv_p, wq_p, wk_p, wp_p))

        bv_sb = const.tile([128, 3], F32)
        nc.scalar.dma_start(bv_sb[:], bv_p.ap().rearrange("(oc p) -> p oc",
                                                          p=128))
        bq_row = const.tile([1, C], F32R)
        nc.scalar.dma_start(bq_row[:], bq_p.ap()[:, :])
        bk_row = const.tile([1, C], F32R)
        nc.scalar.dma_start(bk_row[:], bk_p.ap()[:, :])

        bp_bc = const.tile([128, C], F32)
        bp_ap = bp_p.ap()
        nc.scalar.dma_start(bp_bc[:], bass.AP(
            tensor=bp_ap.tensor, offset=bp_ap.offset, ap=[[0, 128], *bp_ap.ap]))

        id48 = const.tile([48, 48], F32)
        make_identity(nc, id48[:])

        state = {}

        # ---- v projection + Gram accumulation, streamed over N -----------
        def emit_vg(b, steps):
            vT_b = big.tile([128, 3, N], BF16, tag="vT", name=f"vT{b}")
            g_ps_b = [psum.tile([128, C + 1], F32, tag="g", bufs=5,
                                name=f"g{b}_{i}") for i in range(3)]
            state[b] = (vT_b, g_ps_b)
            for w in range(NWIN):
                if steps:
                    steps.pop(0)()
                xw = xin.tile([128, 3, NW], BF16, tag="xw")
                nc.sync.dma_start(xw[:], xt_p.ap()[b, w])
                xnw = xnp.tile([128, 4, C + 1], BF16, tag="xnw")
                nc.sync.dma_start(xnw[:], xn_p.ap()[b, w])

                for oc in range(3):
                    v_ps = psum.tile([128, NW], F32, tag="v", bufs=2,
                                     name="v_ps")
                    for kc in range(3):
                        nc.tensor.matmul(
                            v_ps[:],
                            wv_sb[:, kc, oc * 128:(oc + 1) * 128],
                            xw[:, kc, :],
                            start=(kc == 0), stop=(kc == 2),
                        )
                    nc.scalar.activation(
                        vT_b[:, oc, w * NW:(w + 1) * NW], v_ps[:],
                        mybir.ActivationFunctionType.Identity,
                        bias=bv_sb[:, oc:oc + 1], scale=1.0,
                    )

                for ns in range(4):
                    t_chunk = 4 * w + ns
                    for oc in range(3):
                        nc.tensor.matmul(
                            g_ps_b[oc][:],
                            xnw[:, ns, oc * 128:(oc + 1) * 128],
                            xnw[:, ns, :],
                            start=(t_chunk == 0), stop=(t_chunk == NCHUNK - 1),
                        )
            while steps:
                steps.pop(0)()

        # ---- S = softmax logits chain, chopped into interleavable steps --
        def make_schain(b):
            st = {}

            def s1():
                _, g_ps_b = state[b]
                g_sb = sm.tile([128, 3, C + 1], BF16, tag="g_sb")
                for oc in range(3):
                    nc.scalar.activation(
                        g_sb[:, oc, :], g_ps_b[oc][:],
                        mybir.ActivationFunctionType.Identity,
                        bias=0.0, scale=1.0)
                st["g"] = g_sb

            def s2():
                g_sb = st["g"]
                t_sb = sm.tile([128, 3, C], BF16, tag="t_sb")
                for c1 in range(3):
                    t_ps = psum.tile([128, C], F32, tag="chain", bufs=1,
                                     name="t_ps")
                    for kc2 in range(3):
                        nc.tensor.matmul(
                            t_ps[:],
                            g_sb[:, kc2, c1 * 128:(c1 + 1) * 128],
                            wk_sb[:, kc2, :],
                            start=(kc2 == 0), stop=(kc2 == 2),
                        )
                    nc.scalar.activation(
                        t_sb[:, c1, :], t_ps[:],
                        mybir.ActivationFunctionType.Identity,
                        bias=0.0, scale=1.0)
                st["t"] = t_sb

            def s3():
                g_sb = st["g"]
                uv = []
                for wsb in (wq_sb, wk_sb):
                    r_ps = psum.tile([1, C], F32, tag="chain", bufs=1,
                                     name="r_ps")
                    for kc in range(3):
                        nc.tensor.matmul(
                            r_ps[:], g_sb[:, kc, C:C + 1], wsb[:, kc, :],
                            start=(kc == 0), stop=(kc == 2),
                        )
                    r_sb = sm.tile([1, C], F32R, tag=f"uv{len(uv)}",
                                   name="r_sb")
                    nc.vector.tensor_copy(r_sb[:], r_ps[:])
                    uv.append(r_sb)
                vn = sm.tile([1, C], F32R, tag="vn")
                nc.vector.tensor_scalar_mul(vn[:], bk_row[:], float(N))
                nc.vector.tensor_add(vn[:], vn[:], uv[1][:])
                st["u"], st["vn"] = uv[0], vn

            def s4():
                t_sb, u_sb, vn_sb = st["t"], st["u"], st["vn"]
                s_ps = psum.tile([48, H, 48], F32, tag="chain", bufs=1,
                                 name="s_ps")
                for h in range(H):
                    hsl = slice(48 * h, 48 * (h + 1))
                    for kc1 in range(3):
                        nc.tensor.matmul(
                            s_ps[:, h, :], wq_sb[:, kc1, hsl],
                            t_sb[:, kc1, hsl],
                            start=(kc1 == 0), stop=False,
                        )
                    nc.tensor.matmul(s_ps[:, h, :], u_sb[:, hsl],
                                     bk_row[:, hsl], start=False, stop=False)
                    nc.tensor.matmul(s_ps[:, h, :], bq_row[:, hsl],
                                     vn_sb[:, hsl], start=False, stop=True)
                p_all = sm.tile([48, H, 48], F32, tag="p_all")
                nc.scalar.activation(
                    p_all[:], s_ps[:], mybir.ActivationFunctionType.Exp,
                    bias=0.0, scale=SCALE)
                zsum = sm.tile([48, H], F32, tag="zsum")
                nc.vector.reduce_sum(zsum[:], p_all[:],
                                     axis=mybir.AxisListType.X)
                zrec = sm.tile([48, H], F32, tag="zrec")
                nc.vector.reciprocal(zrec[:], zsum[:])
                attn = sm.tile([48, H, 48], F32, tag="attn")
                for h in range(H):
                    nc.vector.tensor_scalar_mul(
                        attn[:, h, :], p_all[:, h, :], zrec[:, h:h + 1])
                st["attn"] = attn

            def s5():
                attn = st["attn"]
                # transpose two heads per op: attn[:, h:h+2, :] is [48, 96];
                # its transpose stacks attn_h^T (rows 0:48) over
                # attn_{h+1}^T (rows 48:96)
                tr_ps = psum.tile([96, H // 2, 48], F32, tag="chain", bufs=1,
                                  name="tr_ps")
                for hp in range(H // 2):
                    nc.tensor.transpose(tr_ps[:, hp, :],
                                        attn[:, 2 * hp:2 * hp + 2, :],
                                        id48[:])
                attn_t = sm.tile([96, H // 2, 48], BF16, tag="attn_t")
                nc.scalar.activation(
                    attn_t[:], tr_ps[:],
                    mybir.ActivationFunctionType.Identity,
                    bias=0.0, scale=1.0)
                st["attn_t"] = attn_t

            def s6():
                attn_t = st["attn_t"]
                bd_b = bd[b % 2]
                for h in range(H):
                    hp, e = h // 2, h % 2
                    c0 = 48 * h
                    dj = 0
                    while dj < 48:
                        kc, off = (c0 + dj) // 128, (c0 + dj) % 128
                        cnt = min(48 - dj, 128 - off)
                        nc.gpsimd.dma_start(
                            bd_b[kc][off:off + cnt, c0:c0 + 48],
                            attn_t[48 * e + dj:48 * e + dj + cnt, hp, :])
                        dj += cnt

            return [s1, s2, s3, s4, s5, s6]

        # ---- stage 5 (attn @ v, sparse) + stage 6 (output projection) ----
        def emit_out(b, steps):
            vT_b, _ = state[b]
            at = big1.tile([128, C * NCHUNK], BF16, tag="at", name=f"at{b}")
            atv = at[:].rearrange("p (d h t) -> p h d t", h=H, t=NCHUNK)
            atr = at[:].rearrange("p (r j) -> p r j", j=3)
            bd_b = bd[b % 2]

            for t in range(NCHUNK):
                if t % 3 == 0 and steps:
                    steps.pop(0)()
                at_ps = psum.tile([128, C], F32, tag="g", bufs=5,
                                  name="at_ps")
                for kc in range(3):
                    nc.tensor.matmul(
                        at_ps[:],
                        vT_b[:, kc, t * 128:(t + 1) * 128],
                        bd_b[kc][:],
                        start=(kc == 0), stop=(kc == 2),
                    )
                # 2-op evacuation (DVE+ACT): strided writes are forced by
                # the at layout (stage 6's stride-3 slicing requires it);
                # gpsimd cannot read PSUM, so it takes no share.
                nc.vector.tensor_copy(atv[:, 0:5, :, t], at_ps[:, 0:240])
                nc.scalar.activation(
                    atv[:, 5:8, :, t], at_ps[:, 240:384],
                    mybir.ActivationFunctionType.Identity, bias=0.0, scale=1.0)

            # stage 6: f32r x f32r (bf16 lhsT with f32r rhs is illegal,
            # and bf16 at would make the strided evacuation 2.5x slower)
            for g in range(NG):
                if steps:
                    steps.pop(0)()
                y_sb = yout.tile([128, 4, C], BF16, tag="ysb")
                for j in range(4):
                    rw = 4 * g + j
                    y_ps = psum.tile([128, C], F32, tag="g", bufs=5,
                                     name="y_ps")
                    for jj in range(3):
                        nc.tensor.matmul(
                            y_ps[:],
                            atr[:, rw * 128:(rw + 1) * 128, jj],
                            wp_sb[:, jj, :],
                            start=(jj == 0), stop=(jj == 2),
                        )
                    nc.vector.tensor_add(y_sb[:, j, :], y_ps[:], bp_bc[:])
                nc.sync.dma_start(outp.ap()[b, g], y_sb[:])
            while steps:
                steps.pop(0)()

        emit_vg(0, [])
        emit_vg(1, make_schain(0))
        emit_out(0, make_schain(1))
        emit_out(1, [])

    nc.compile()
    return nc


_CACHE = {}


def prepare_in_maps(x, conv_w, conv_b, wq, bq, wkv, bkv, wp, bp):
    import ml_dtypes

    bf16 = ml_dtypes.bfloat16
    f32 = np.float32
    x = np.ascontiguousarray(x, dtype=f32)

    # fold the 1x1 conv into the projections (host-side weight prep)
    wk_w, wv_w = wkv[:C], wkv[C:]
    bk_b, bv_b = bkv[:C], bkv[C:]
    wqT = np.ascontiguousarray((wq @ conv_w).T, dtype=bf16)
    wkT = np.ascontiguousarray((wk_w @ conv_w).T, dtype=bf16)
    wvT = np.ascontiguousarray((wv_w @ conv_w).T, dtype=bf16)
    wpT = np.ascontiguousarray(wp.T, dtype=bf16)
    bq_e = np.ascontiguousarray((bq + wq @ conv_b).reshape(1, C), dtype=f32)
    bk_e = np.ascontiguousarray((bk_b + wk_w @ conv_b).reshape(1, C), dtype=f32)
    bv_e = np.ascontiguousarray(bv_b + wv_w @ conv_b, dtype=f32)
    bp_c = np.ascontiguousarray(bp, dtype=f32)

    xb = x.astype(bf16)
    # window-tiled transposed x: [B, w, p, kc, n]
    xt_t = np.ascontiguousarray(
        xb.transpose(0, 2, 1).reshape(B, 3, 128, NWIN, NW)
        .transpose(0, 3, 2, 1, 4))
    # window-tiled natural x with ones column: [B, w, p, ns, c]
    xn = np.concatenate([xb, np.ones((B, N, 1), dtype=bf16)], axis=2)
    xn_t = np.ascontiguousarray(
        xn.reshape(B, NWIN, 4, 128, C + 1).transpose(0, 1, 3, 2, 4))

    in_maps = []
    for c in range(N_CORES):
        in_maps.append({
            "xt": xt_t[c * BPC:(c + 1) * BPC],
            "xn": xn_t[c * BPC:(c + 1) * BPC],
            "wqT": wqT, "wkT": wkT, "wvT": wvT, "wpT": wpT,
            "bq": bq_e, "bk": bk_e, "bv": bv_e, "bp": bp_c,
        })

    return in_maps


def kernel(x, conv_w, conv_b, wq, bq, wkv, bkv, wp, bp):
    _install_ntff_hook()
    in_maps = prepare_in_maps(x, conv_w, conv_b, wq, bq, wkv, bkv, wp, bp)
    if "nc" not in _CACHE:
        _CACHE["nc"] = build()
    nc = _CACHE["nc"]
    res = run_bass_kernel_spmd(nc, in_maps, core_ids=list(range(N_CORES)))
    # untile: [BPC, NG, 128, 4, C] -> [BPC, N, C]
    outs = []
    for c in range(N_CORES):
        o = np.asarray(res.results[c]["out"])
        outs.append(o.transpose(0, 1, 3, 2, 4).reshape(BPC, N, C))
    return np.concatenate(outs, axis=0).astype(np.float32)


# revision 19
# speedup vs baseline: 1.5015x; 1.5015x over previous
"""Trainium2 Bass kernel for the channel-attention module.

Reference computation (B=16, N=4096, C=384, H=8, D=48):
    x_in = x @ conv_w.T + conv_b                      # 1x1 conv == linear
    q    = (x_in @ wq.T + bq)  -> [B,H,D,N]
    k, v = (x_in @ wkv.T + bkv) -> 2x [B,H,D,N]
    attn = softmax((q * N**-0.5) @ k^T, axis=-1)      # [B,H,D,D] (over N!)
    out  = attn @ v                                   # [B,H,D,N]
    out  = out.transpose(0,2,1,3).reshape(B,N,C)      # verbatim torch layout
    y    = out @ wp.T + bp

Strategy: pure data parallelism over B across 8 NeuronCores (2 batches per
core), no collectives.  The conv is folded into the q/k/v projections on the
host (w_eff = (w @ conv_w).T, b_eff = b + w @ conv_b).

q and k are never materialized: since the attention logits contract over N,
S_h = (X wq + 1 bq^T)_h^T (X wk + 1 bk^T)_h
    = wq_h^T G wk_h + u_h (x) bk_h + bq_h (x) (v_h + N bk_h)
with G = X^T X (Gram matrix, accumulated on-chip in PSUM), s = X^T 1 (folded
into G as an extra column via a ones-column appended to x on the host),
u = wq^T s, v = wk^T s.  The rank-1 bias terms are added with K=1 matmuls.

The awkward transpose(0,2,1,3).reshape is handled exactly with 128-element
flat blocks: flat index (di, h, n) -> block u = 256*di + 32*h + n//128 with
intra-block offset c' = n%128.  Stage 5 (attn @ v) produces AT[c', u] tiles
through a block-diagonal attn^T matrix; stage 6 reads columns u = 3*r + j
(stride-3 APs) as the K=128 slices of the final projection.

Perf structure (final, 199us -> 171us):
  - inputs/outputs host-tiled so every window DMA is contiguous per
    partition (~3KB descriptors, ~330 GB/s); output is bf16, untiled on
    the host.
  - bd zeros via on-chip memset (no HBM dependency before PE warmup).
  - issue order: b0 v/G | b1 v/G + b0 S-chain interleaved | b0 st5/6 + b1
    S-chain interleaved | b1 st5/6 -- the serial softmax chain hides under
    the other batch's dense matmul stream (PE executes in issue order).
  - stage 6 stays f32r x f32r: bf16 lhsT with f32r rhs is an illegal mix,
    and a bf16 `at` would hit the 2.5x-slower 16-bit strided-write path in
    the evacuation.  The at evacuation (strided writes forced by stage 6's
    stride-3 slicing) is 2 ops (DVE 240 + ACT 144 cols) because per-op
    fixed overhead is ~200-350ns.
  - attn^T via 4 paired PE transposes ([48,96] -> [96,48] stacks two
    heads); scatter into bd on the idle gpsimd SWDGE queue; x windows on
    the SP queue; weights on the ACT queue; y DMAs batched 4 chunks.
  - PSUM tag rings: v(+warmup):2 banks, g(G accum + at_ps + y_ps):5,
    chain(t/r/s/tr):1 = 8; at/y sharing g's ring is deadlock-safe because
    every slot-reuse wait points at an earlier-issued instruction.
  - Known-bad variants (measured): sparse stage-5 segments + multi-op evac
    (249us -- evac op overhead dominates); transposed stage 6 (illegal
    dtype mix); xnw on ACT / scatter on SP / ACT-heavy evac (+6us each).
    LDWEIGHTS is never elided (1 per matmul), so same-lhsT tricks (G
    symmetry, stage-5 column sparsity) lose to the weight-load tax.
"""

import sys
import types
from contextlib import ExitStack

import numpy as np

import concourse.bass as bass
import concourse.tile as tile
from concourse import bacc, mybir
from concourse.bass_utils import run_bass_kernel_spmd
from concourse.masks import make_identity

B, N, C, H, D = 16, 4096, 384, 8, 48
N_CORES = 8
BPC = B // N_CORES          # batches per core
NW = 512                    # token window for the v projection
NWIN = N // NW              # 8 windows
NCHUNK = N // 128           # 32 token chunks of 128
NG = NCHUNK // 4            # 8 output DMA groups
SCALE = float(N) ** -0.5    # 1/64
F32 = mybir.dt.float32
F32R = mybir.dt.float32r
BF16 = mybir.dt.bfloat16

def _install_ntff_hook():
    """The agent image's antenv lacks axon_hooks, so trn_boot's NTFF hook
    registration degrades silently and trace=True would crash.  Recreate the
    module and register the ctypes hook so profiling works."""
    try:
        import antenv

        if "antenv.axon_hooks" in sys.modules:
            return
        mod = types.ModuleType("antenv.axon_hooks")
        mod._hook = None
        mod.set_axon_ntff_profile_hook = lambda h: setattr(mod, "_hook", h)
        mod.get_axon_ntff_profile_hook = lambda: mod._hook
        sys.modules["antenv.axon_hooks"] = mod
        antenv.axon_hooks = mod
        from trn_agent_boot.trn_boot import _ntff_profile_via_ctypes

        mod.set_axon_ntff_profile_hook(
            _ntff_profile_via_ctypes("/opt/axon/libaxon_pjrt.so")
        )
    except Exception:
        pass


def build():
    nc = bacc.Bacc("TRN2", target_bir_lowering=False, debug=False,
                   num_devices=N_CORES)

    # Host-tiled inputs: every [128, ...] tile is contiguous per partition.
    xt_p = nc.declare_dram_parameter("xt", [BPC, NWIN, 128, 3, NW], BF16,
                                     isOutput=False)
    xn_p = nc.declare_dram_parameter("xn", [BPC, NWIN, 128, 4, C + 1], BF16,
                                     isOutput=False)
    wq_p = nc.declare_dram_parameter("wqT", [C, C], BF16, isOutput=False)
    wk_p = nc.declare_dram_parameter("wkT", [C, C], BF16, isOutput=False)
    wv_p = nc.declare_dram_parameter("wvT", [C, C], BF16, isOutput=False)
    wp_p = nc.declare_dram_parameter("wpT", [C, C], F32R, isOutput=False)
    bq_p = nc.declare_dram_parameter("bq", [1, C], F32R, isOutput=False)
    bk_p = nc.declare_dram_parameter("bk", [1, C], F32R, isOutput=False)
    bv_p = nc.declare_dram_parameter("bv", [C], F32, isOutput=False)
    bp_p = nc.declare_dram_parameter("bp", [C], F32, isOutput=False)
    outp = nc.declare_dram_parameter("out", [BPC, NG, 128, 4, C], BF16,
                                     isOutput=True)

    with tile.TileContext(nc) as tc, ExitStack() as ctx:
        const = ctx.enter_context(tc.tile_pool(name="const", bufs=1))
        xin = ctx.enter_context(tc.tile_pool(name="xin", bufs=6))
        xnp = ctx.enter_context(tc.tile_pool(name="xnp", bufs=6))
        big = ctx.enter_context(tc.tile_pool(name="big", bufs=2))
        big1 = ctx.enter_context(tc.tile_pool(name="big1", bufs=1))
        sm = ctx.enter_context(tc.tile_pool(name="sm", bufs=2))
        yout = ctx.enter_context(tc.tile_pool(name="yout", bufs=3))
        psum = ctx.enter_context(tc.tile_pool(name="ps", bufs=1, space="PSUM"))

        # ---- block-diag tiles (two sets, one per batch) zeroed on-chip ----
        bd = [[const.tile([128, C], BF16, tag=f"bd{s}_{i}",
                          name=f"bd{s}_{i}") for i in range(3)]
              for s in range(2)]
        for i in range(3):
            nc.vector.memset(bd[0][i][:], 0.0)
        for i in range(3):
            nc.gpsimd.memset(bd[1][i][:], 0.0)

        # PE warmup on the zeroed tiles (keeps HAM busy until x lands)
        warm_ps = psum.tile([128, NW], F32, tag="v", bufs=2, name="warm_ps")
        for i in range(5):
            nc.tensor.matmul(warm_ps[:, 0:256], bd[0][i % 3][:, 0:128],
                             bd[0][i % 3][:, 0:256],
                             start=(i == 0), stop=(i == 4))
        warm_sb = const.tile([128, 1], BF16)
        nc.vector.tensor_copy(warm_sb[:], warm_ps[:, 0:1])
        nc.gpsimd.dma_start(outp.ap()[0, 0, :, 0, 0:1], warm_sb[:])

        # ---- constants on the ACT DMA queue (SP queue streams x) ----------
        def load_w(param):
            t = const.tile([128, 3, C], param.dtype, tag=f"w_{param.name}")
            nc.scalar.dma_start(
                t[:], param.ap().rearrange("(kc p) o -> p kc o", p=128))
            return t

        wv_sb, wq_sb, wk_sb, wp_sb = (load_w(p) for p in
                                      (wv_p, wq_p, wk_p, wp_p))

        bv_sb = const.tile([128, 3], F32)
        nc.scalar.dma_start(bv_sb[:], bv_p.ap().rearrange("(oc p) -> p oc",
                                                          p=128))
        bq_row = const.tile([1, C], F32R)
        nc.scalar.dma_start(bq_row[:], bq_p.ap()[:, :])
        bk_row = const.tile([1, C], F32R)
        nc.scalar.dma_start(bk_row[:], bk_p.ap()[:, :])

        bp_bc = const.tile([128, C], F32)
        bp_ap = bp_p.ap()
        nc.scalar.dma_start(bp_bc[:], bass.AP(
            tensor=bp_ap.tensor, offset=bp_ap.offset, ap=[[0, 128], *bp_ap.ap]))

        id48 = const.tile([48, 48], F32)
        make_identity(nc, id48[:])

        state = {}

        # ---- v projection + Gram accumulation, streamed over N -----------
        def emit_vg(b, steps):
            vT_b = big.tile([128, 3, N], BF16, tag="vT", name=f"vT{b}")
            g_ps_b = [psum.tile([128, C + 1], F32, tag="g", bufs=5,
                                name=f"g{b}_{i}") for i in range(3)]
            state[b] = (vT_b, g_ps_b)
            for w in range(NWIN):
                if steps:
                    steps.pop(0)()
                xw = xin.tile([128, 3, NW], BF16, tag="xw")
                nc.sync.dma_start(xw[:], xt_p.ap()[b, w])
                xnw = xnp.tile([128, 4, C + 1], BF16, tag="xnw")
                nc.sync.dma_start(xnw[:], xn_p.ap()[b, w])

                for oc in range(3):
                    v_ps = psum.tile([128, NW], F32, tag="v", bufs=2,
                                     name="v_ps")
                    for kc in range(3):
                        nc.tensor.matmul(
                            v_ps[:],
                            wv_sb[:, kc, oc * 128:(oc + 1) * 128],
                            xw[:, kc, :],
                            start=(kc == 0), stop=(kc == 2),
                        )
                    nc.scalar.activation(
                        vT_b[:, oc, w * NW:(w + 1) * NW], v_ps[:],
                        mybir.ActivationFunctionType.Identity,
                        bias=bv_sb[:, oc:oc + 1], scale=1.0,
                    )

                for ns in range(4):
                    t_chunk = 4 * w + ns
                    for oc in range(3):
                        nc.tensor.matmul(
                            g_ps_b[oc][:],
                            xnw[:, ns, oc * 128:(oc + 1) * 128],
                            xnw[:, ns, :],
                            start=(t_chunk == 0), stop=(t_chunk == NCHUNK - 1),
                        )
            while steps:
                steps.pop(0)()

        # ---- S = softmax logits chain, chopped into interleavable steps --
        def make_schain(b):
            st = {}

            def s1():
                _, g_ps_b = state[b]
                g_sb = sm.tile([128, 3, C + 1], BF16, tag="g_sb")
                for oc in range(3):
                    nc.scalar.activation(
                        g_sb[:, oc, :], g_ps_b[oc][:],
                        mybir.ActivationFunctionType.Identity,
                        bias=0.0, scale=1.0)
                st["g"] = g_sb

            def s2():
                g_sb = st["g"]
                t_sb = sm.tile([128, 3, C], BF16, tag="t_sb")
                for c1 in range(3):
                    t_ps = psum.tile([128, C], F32, tag="chain", bufs=1,
                                     name="t_ps")
                    for kc2 in range(3):
                        nc.tensor.matmul(
                            t_ps[:],
                            g_sb[:, kc2, c1 * 128:(c1 + 1) * 128],
                            wk_sb[:, kc2, :],
                            start=(kc2 == 0), stop=(kc2 == 2),
                        )
                    nc.scalar.activation(
                        t_sb[:, c1, :], t_ps[:],
                        mybir.ActivationFunctionType.Identity,
                        bias=0.0, scale=1.0)
                st["t"] = t_sb

            def s3():
                g_sb = st["g"]
                uv = []
                for wsb in (wq_sb, wk_sb):
                    r_ps = psum.tile([1, C], F32, tag="chain", bufs=1,
                                     name="r_ps")
                    for kc in range(3):
                        nc.tensor.matmul(
                            r_ps[:], g_sb[:, kc, C:C + 1], wsb[:, kc, :],
                            start=(kc == 0), stop=(kc == 2),
                        )
                    r_sb = sm.tile([1, C], F32R, tag=f"uv{len(uv)}",
                                   name="r_sb")
                    nc.vector.tensor_copy(r_sb[:], r_ps[:])
                    uv.append(r_sb)
                vn = sm.tile([1, C], F32R, tag="vn")
                nc.vector.tensor_scalar_mul(vn[:], bk_row[:], float(N))
                nc.vector.tensor_add(vn[:], vn[:], uv[1][:])
                st["u"], st["vn"] = uv[0], vn

            def s4():
                t_sb, u_sb, vn_sb = st["t"], st["u"], st["vn"]
                s_ps = psum.tile([48, H, 48], F32, tag="chain", bufs=1,
                                 name="s_ps")
                for h in range(H):
                    hsl = slice(48 * h, 48 * (h + 1))
                    for kc1 in range(3):
                        nc.tensor.matmul(
                            s_ps[:, h, :], wq_sb[:, kc1, hsl],
                            t_sb[:, kc1, hsl],
                            start=(kc1 == 0), stop=False,
                        )
                    nc.tensor.matmul(s_ps[:, h, :], u_sb[:, hsl],
                                     bk_row[:, hsl], start=False, stop=False)
                    nc.tensor.matmul(s_ps[:, h, :], bq_row[:, hsl],
                                     vn_sb[:, hsl], start=False, stop=True)
                p_all = sm.tile([48, H, 48], F32, tag="p_all")
                nc.scalar.activation(
                    p_all[:], s_ps[:], mybir.ActivationFunctionType.Exp,
                    bias=0.0, scale=SCALE)
                zsum = sm.tile([48, H], F32, tag="zsum")
                nc.vector.reduce_sum(zsum[:], p_all[:],
                                     axis=mybir.AxisListType.X)
                zrec = sm.tile([48, H], F32, tag="zrec")
                nc.vector.reciprocal(zrec[:], zsum[:])
                attn = sm.tile([48, H, 48], F32, tag="attn")
                for h in range(H):
                    nc.vector.tensor_scalar_mul(
                        attn[:, h, :], p_all[:, h, :], zrec[:, h:h + 1])
                st["attn"] = attn

            def s5():
                attn = st["attn"]
                # transpose two heads per op: attn[:, h:h+2, :] is [48, 96];
                # its transpose stacks attn_h^T (rows 0:48) over
                # attn_{h+1}^T (rows 48:96)
                tr_ps = psum.tile([96, H // 2, 48], F32, tag="chain", bufs=1,
                                  name="tr_ps")
                for hp in range(H // 2):
                    nc.tensor.transpose(tr_ps[:, hp, :],
                                        attn[:, 2 * hp:2 * hp + 2, :],
                                        id48[:])
                attn_t = sm.tile([96, H // 2, 48], BF16, tag="attn_t")
                nc.scalar.activation(
                    attn_t[:], tr_ps[:],
                    mybir.ActivationFunctionType.Identity,
                    bias=0.0, scale=1.0)
                st["attn_t"] = attn_t

            def s6():
                attn_t = st["attn_t"]
                bd_b = bd[b % 2]
                for h in range(H):
                    hp, e = h // 2, h % 2
                    c0 = 48 * h
                    dj = 0
                    while dj < 48:
                        kc, off = (c0 + dj) // 128, (c0 + dj) % 128
                        cnt = min(48 - dj, 128 - off)
                        nc.gpsimd.dma_start(
                            bd_b[kc][off:off + cnt, c0:c0 + 48],
                            attn_t[48 * e + dj:48 * e + dj + cnt, hp, :])
                        dj += cnt

            return [s1, s2, s3, s4, s5, s6]

        # ---- stage 5 (attn @ v, sparse) + stage 6 (output projection) ----
        def emit_out(b, steps):
            vT_b, _ = state[b]
            at = big1.tile([128, C * NCHUNK], F32R, tag="at", name=f"at{b}")
            atv = at[:].rearrange("p (d h t) -> p h d t", h=H, t=NCHUNK)
            atr = at[:].rearrange("p (r j) -> p r j", j=3)
            bd_b = bd[b % 2]

            for t in range(NCHUNK):
                if t % 3 == 0 and steps:
                    steps.pop(0)()
                at_ps = psum.tile([128, C], F32, tag="g", bufs=5,
                                  name="at_ps")
                for kc in range(3):
                    nc.tensor.matmul(
                        at_ps[:],
                        vT_b[:, kc, t * 128:(t + 1) * 128],
                        bd_b[kc][:],
                        start=(kc == 0), stop=(kc == 2),
                    )
                # 2-op evacuation (DVE+ACT): strided writes are forced by
                # the at layout (stage 6's stride-3 slicing requires it);
                # gpsimd cannot read PSUM, so it takes no share.
                nc.vector.tensor_copy(atv[:, 0:5, :, t], at_ps[:, 0:240])
                nc.scalar.activation(
                    atv[:, 5:8, :, t], at_ps[:, 240:384],
                    mybir.ActivationFunctionType.Identity, bias=0.0, scale=1.0)

            # stage 6: f32r x f32r (bf16 lhsT with f32r rhs is illegal,
            # and bf16 at would make the strided evacuation 2.5x slower)
            for g in range(NG):
                if steps:
                    steps.pop(0)()
                y_sb = yout.tile([128, 4, C], BF16, tag="ysb")
                for j in range(4):
                    rw = 4 * g + j
                    y_ps = psum.tile([128, C], F32, tag="g", bufs=5,
                                     name="y_ps")
                    for jj in range(3):
                        nc.tensor.matmul(
                            y_ps[:],
                            atr[:, rw * 128:(rw + 1) * 128, jj],
                            wp_sb[:, jj, :],
                            start=(jj == 0), stop=(jj == 2),
                        )
                    nc.vector.tensor_add(y_sb[:, j, :], y_ps[:], bp_bc[:])
                nc.sync.dma_start(outp.ap()[b, g], y_sb[:])
            while steps:
                steps.pop(0)()

        emit_vg(0, [])
        emit_vg(1, make_schain(0))
        emit_out(0, make_schain(1))
        emit_out(1, [])

    nc.compile()
    return nc


_CACHE = {}


def prepare_in_maps(x, conv_w, conv_b, wq, bq, wkv, bkv, wp, bp):
    import ml_dtypes

    bf16 = ml_dtypes.bfloat16
    f32 = np.float32
    x = np.ascontiguousarray(x, dtype=f32)

    # fold the 1x1 conv into the projections (host-side weight prep)
    wk_w, wv_w = wkv[:C], wkv[C:]
    bk_b, bv_b = bkv[:C], bkv[C:]
    wqT = np.ascontiguousarray((wq @ conv_w).T, dtype=bf16)
    wkT = np.ascontiguousarray((wk_w @ conv_w).T, dtype=bf16)
    wvT = np.ascontiguousarray((wv_w @ conv_w).T, dtype=bf16)
    wpT = np.ascontiguousarray(wp.T, dtype=f32)
    bq_e = np.ascontiguousarray((bq + wq @ conv_b).reshape(1, C), dtype=f32)
    bk_e = np.ascontiguousarray((bk_b + wk_w @ conv_b).reshape(1, C), dtype=f32)
    bv_e = np.ascontiguousarray(bv_b + wv_w @ conv_b, dtype=f32)
    bp_c = np.ascontiguousarray(bp, dtype=f32)

    xb = x.astype(bf16)
    # window-tiled transposed x: [B, w, p, kc, n]
    xt_t = np.ascontiguousarray(
        xb.transpose(0, 2, 1).reshape(B, 3, 128, NWIN, NW)
        .transpose(0, 3, 2, 1, 4))
    # window-tiled natural x with ones column: [B, w, p, ns, c]
    xn = np.concatenate([xb, np.ones((B, N, 1), dtype=bf16)], axis=2)
    xn_t = np.ascontiguousarray(
        xn.reshape(B, NWIN, 4, 128, C + 1).transpose(0, 1, 3, 2, 4))

    in_maps = []
    for c in range(N_CORES):
        in_maps.append({
            "xt": xt_t[c * BPC:(c + 1) * BPC],
            "xn": xn_t[c * BPC:(c + 1) * BPC],
            "wqT": wqT, "wkT": wkT, "wvT": wvT, "wpT": wpT,
            "bq": bq_e, "bk": bk_e, "bv": bv_e, "bp": bp_c,
        })

    return in_maps


def kernel(x, conv_w, conv_b, wq, bq, wkv, bkv, wp, bp):
    _install_ntff_hook()
    in_maps = prepare_in_maps(x, conv_w, conv_b, wq, bq, wkv, bkv, wp, bp)
    if "nc" not in _CACHE:
        _CACHE["nc"] = build()
    nc = _CACHE["nc"]
    res = run_bass_kernel_spmd(nc, in_maps, core_ids=list(range(N_CORES)))
    # untile: [BPC, NG, 128, 4, C] -> [BPC, N, C]
    outs = []
    for c in range(N_CORES):
        o = np.asarray(res.results[c]["out"])
        outs.append(o.transpose(0, 1, 3, 2, 4).reshape(BPC, N, C))
    return np.concatenate(outs, axis=0).astype(np.float32)


# revision 20
# speedup vs baseline: 1.5063x; 1.0032x over previous
"""Trainium2 Bass kernel for the channel-attention module.

Reference computation (B=16, N=4096, C=384, H=8, D=48):
    x_in = x @ conv_w.T + conv_b                      # 1x1 conv == linear
    q    = (x_in @ wq.T + bq)  -> [B,H,D,N]
    k, v = (x_in @ wkv.T + bkv) -> 2x [B,H,D,N]
    attn = softmax((q * N**-0.5) @ k^T, axis=-1)      # [B,H,D,D] (over N!)
    out  = attn @ v                                   # [B,H,D,N]
    out  = out.transpose(0,2,1,3).reshape(B,N,C)      # verbatim torch layout
    y    = out @ wp.T + bp

Strategy: pure data parallelism over B across 8 NeuronCores (2 batches per
core), no collectives.  The conv is folded into the q/k/v projections on the
host (w_eff = (w @ conv_w).T, b_eff = b + w @ conv_b).

q and k are never materialized: since the attention logits contract over N,
S_h = (X wq + 1 bq^T)_h^T (X wk + 1 bk^T)_h
    = wq_h^T G wk_h + u_h (x) bk_h + bq_h (x) (v_h + N bk_h)
with G = X^T X (Gram matrix, accumulated on-chip in PSUM), s = X^T 1 (folded
into G as an extra column via a ones-column appended to x on the host),
u = wq^T s, v = wk^T s.  The rank-1 bias terms are added with K=1 matmuls.

The awkward transpose(0,2,1,3).reshape is handled exactly with 128-element
flat blocks: flat index (di, h, n) -> block u = 256*di + 32*h + n//128 with
intra-block offset c' = n%128.  Stage 5 (attn @ v) produces AT[c', u] tiles
through a block-diagonal attn^T matrix; stage 6 reads columns u = 3*r + j
(stride-3 APs) as the K=128 slices of the final projection.

Perf structure (final, 199us -> 171us):
  - inputs/outputs host-tiled so every window DMA is contiguous per
    partition (~3KB descriptors, ~330 GB/s); output is bf16, untiled on
    the host.
  - bd zeros via on-chip memset (no HBM dependency before PE warmup).
  - issue order: b0 v/G | b1 v/G + b0 S-chain interleaved | b0 st5/6 + b1
    S-chain interleaved | b1 st5/6 -- the serial softmax chain hides under
    the other batch's dense matmul stream (PE executes in issue order).
  - stage 6 stays f32r x f32r: bf16 lhsT with f32r rhs is an illegal mix,
    and a bf16 `at` would hit the 2.5x-slower 16-bit strided-write path in
    the evacuation.  The at evacuation (strided writes forced by stage 6's
    stride-3 slicing) is 2 ops (DVE 240 + ACT 144 cols) because per-op
    fixed overhead is ~200-350ns.
  - attn^T via 4 paired PE transposes ([48,96] -> [96,48] stacks two
    heads); scatter into bd on the idle gpsimd SWDGE queue; x windows on
    the SP queue; weights on the ACT queue; y DMAs batched 4 chunks.
  - PSUM tag rings: v(+warmup):2 banks, g(G accum + at_ps + y_ps):5,
    chain(t/r/s/tr):1 = 8; at/y sharing g's ring is deadlock-safe because
    every slot-reuse wait points at an earlier-issued instruction.
  - Known-bad variants (measured): sparse stage-5 segments + multi-op evac
    (249us -- evac op overhead dominates); transposed stage 6 (illegal
    dtype mix); xnw on ACT / scatter on SP / ACT-heavy evac (+6us each).
    LDWEIGHTS is never elided (1 per matmul), so same-lhsT tricks (G
    symmetry, stage-5 column sparsity) lose to the weight-load tax.
"""

import sys
import types
from contextlib import ExitStack

import numpy as np

import concourse.bass as bass
import concourse.tile as tile
from concourse import bacc, mybir
from concourse.bass_utils import run_bass_kernel_spmd
from concourse.masks import make_identity

B, N, C, H, D = 16, 4096, 384, 8, 48
N_CORES = 8
BPC = B // N_CORES          # batches per core
NW = 512                    # token window for the v projection
NWIN = N // NW              # 8 windows
NCHUNK = N // 128           # 32 token chunks of 128
NG = NCHUNK // 4            # 8 output DMA groups
SCALE = float(N) ** -0.5    # 1/64
F32 = mybir.dt.float32
F32R = mybir.dt.float32r
BF16 = mybir.dt.bfloat16

def _install_ntff_hook():
    """The agent image's antenv lacks axon_hooks, so trn_boot's NTFF hook
    registration degrades silently and trace=True would crash.  Recreate the
    module and register the ctypes hook so profiling works."""
    try:
        import antenv

        if "antenv.axon_hooks" in sys.modules:
            return
        mod = types.ModuleType("antenv.axon_hooks")
        mod._hook = None
        mod.set_axon_ntff_profile_hook = lambda h: setattr(mod, "_hook", h)
        mod.get_axon_ntff_profile_hook = lambda: mod._hook
        sys.modules["antenv.axon_hooks"] = mod
        antenv.axon_hooks = mod
        from trn_agent_boot.trn_boot import _ntff_profile_via_ctypes

        mod.set_axon_ntff_profile_hook(
            _ntff_profile_via_ctypes("/opt/axon/libaxon_pjrt.so")
        )
    except Exception:
        pass


def build():
    nc = bacc.Bacc("TRN2", target_bir_lowering=False, debug=False,
                   num_devices=N_CORES)

    # Host-tiled inputs: every [128, ...] tile is contiguous per partition.
    xt_p = nc.declare_dram_parameter("xt", [BPC, NWIN, 128, 3, NW], BF16,
                                     isOutput=False)
    xn_p = nc.declare_dram_parameter("xn", [BPC, NWIN, 128, 4, C + 1], BF16,
                                     isOutput=False)
    wq_p = nc.declare_dram_parameter("wqT", [C, C], BF16, isOutput=False)
    wk_p = nc.declare_dram_parameter("wkT", [C, C], BF16, isOutput=False)
    wv_p = nc.declare_dram_parameter("wvT", [C, C], BF16, isOutput=False)
    wp_p = nc.declare_dram_parameter("wpT", [C, C], F32R, isOutput=False)
    bq_p = nc.declare_dram_parameter("bq", [1, C], F32R, isOutput=False)
    bk_p = nc.declare_dram_parameter("bk", [1, C], F32R, isOutput=False)
    bv_p = nc.declare_dram_parameter("bv", [C], F32, isOutput=False)
    bp_p = nc.declare_dram_parameter("bp", [C], F32, isOutput=False)
    outp = nc.declare_dram_parameter("out", [BPC, NG, 128, 4, C], BF16,
                                     isOutput=True)

    with tile.TileContext(nc) as tc, ExitStack() as ctx:
        const = ctx.enter_context(tc.tile_pool(name="const", bufs=1))
        xin = ctx.enter_context(tc.tile_pool(name="xin", bufs=6))
        xnp = ctx.enter_context(tc.tile_pool(name="xnp", bufs=6))
        big = ctx.enter_context(tc.tile_pool(name="big", bufs=2))
        big1 = ctx.enter_context(tc.tile_pool(name="big1", bufs=1))
        sm = ctx.enter_context(tc.tile_pool(name="sm", bufs=2))
        yout = ctx.enter_context(tc.tile_pool(name="yout", bufs=3))
        psum = ctx.enter_context(tc.tile_pool(name="ps", bufs=1, space="PSUM"))

        # ---- block-diag tiles (two sets, one per batch) zeroed on-chip ----
        bd = [[const.tile([128, C], BF16, tag=f"bd{s}_{i}",
                          name=f"bd{s}_{i}") for i in range(3)]
              for s in range(2)]
        for i in range(3):
            nc.vector.memset(bd[0][i][:], 0.0)
        for i in range(3):
            nc.gpsimd.memset(bd[1][i][:], 0.0)

        # PE warmup on the zeroed tiles (keeps HAM busy until x lands)
        warm_ps = psum.tile([128, NW], F32, tag="v", bufs=2, name="warm_ps")
        for i in range(5):
            nc.tensor.matmul(warm_ps[:, 0:256], bd[0][i % 3][:, 0:128],
                             bd[0][i % 3][:, 0:256],
                             start=(i == 0), stop=(i == 4))
        warm_sb = const.tile([128, 1], BF16)
        nc.vector.tensor_copy(warm_sb[:], warm_ps[:, 0:1])
        nc.gpsimd.dma_start(outp.ap()[0, 0, :, 0, 0:1], warm_sb[:])

        # ---- constants on the ACT DMA queue (SP queue streams x) ----------
        def load_w(param):
            t = const.tile([128, 3, C], param.dtype, tag=f"w_{param.name}")
            nc.scalar.dma_start(
                t[:], param.ap().rearrange("(kc p) o -> p kc o", p=128))
            return t

        wv_sb, wq_sb, wk_sb, wp_sb = (load_w(p) for p in
                                      (wv_p, wq_p, wk_p, wp_p))

        bv_sb = const.tile([128, 3], F32)
        nc.scalar.dma_start(bv_sb[:], bv_p.ap().rearrange("(oc p) -> p oc",
                                                          p=128))
        bq_row = const.tile([1, C], F32R)
        nc.scalar.dma_start(bq_row[:], bq_p.ap()[:, :])
        bk_row = const.tile([1, C], F32R)
        nc.scalar.dma_start(bk_row[:], bk_p.ap()[:, :])

        bp_bc = const.tile([128, C], F32)
        bp_ap = bp_p.ap()
        nc.scalar.dma_start(bp_bc[:], bass.AP(
            tensor=bp_ap.tensor, offset=bp_ap.offset, ap=[[0, 128], *bp_ap.ap]))

        id48 = const.tile([48, 48], F32)
        make_identity(nc, id48[:])

        state = {}

        # ---- v projection + Gram accumulation, streamed over N -----------
        def emit_vg(b, steps):
            vT_b = big.tile([128, 3, N], BF16, tag="vT", name=f"vT{b}")
            g_ps_b = [psum.tile([128, C + 1], F32, tag="g", bufs=5,
                                name=f"g{b}_{i}") for i in range(3)]
            state[b] = (vT_b, g_ps_b)
            for w in range(NWIN):
                if steps:
                    steps.pop(0)()
                xw = xin.tile([128, 3, NW], BF16, tag="xw")
                nc.sync.dma_start(xw[:], xt_p.ap()[b, w])
                xnw = xnp.tile([128, 4, C + 1], BF16, tag="xnw")
                nc.sync.dma_start(xnw[:], xn_p.ap()[b, w])

                for oc in range(3):
                    v_ps = psum.tile([128, NW], F32, tag="v", bufs=2,
                                     name="v_ps")
                    for kc in range(3):
                        nc.tensor.matmul(
                            v_ps[:],
                            wv_sb[:, kc, oc * 128:(oc + 1) * 128],
                            xw[:, kc, :],
                            start=(kc == 0), stop=(kc == 2),
                        )
                    nc.scalar.activation(
                        vT_b[:, oc, w * NW:(w + 1) * NW], v_ps[:],
                        mybir.ActivationFunctionType.Identity,
                        bias=bv_sb[:, oc:oc + 1], scale=1.0,
                    )

                for ns in range(4):
                    t_chunk = 4 * w + ns
                    for oc in range(3):
                        nc.tensor.matmul(
                            g_ps_b[oc][:],
                            xnw[:, ns, oc * 128:(oc + 1) * 128],
                            xnw[:, ns, :],
                            start=(t_chunk == 0), stop=(t_chunk == NCHUNK - 1),
                        )
            while steps:
                steps.pop(0)()

        # ---- S = softmax logits chain, chopped into interleavable steps --
        def make_schain(b):
            st = {}

            def s1():
                _, g_ps_b = state[b]
                g_sb = sm.tile([128, 3, C + 1], BF16, tag="g_sb")
                for oc in range(3):
                    nc.scalar.activation(
                        g_sb[:, oc, :], g_ps_b[oc][:],
                        mybir.ActivationFunctionType.Identity,
                        bias=0.0, scale=1.0)
                st["g"] = g_sb

            def s2():
                g_sb = st["g"]
                t_sb = sm.tile([128, 3, C], BF16, tag="t_sb")
                for c1 in range(3):
                    t_ps = psum.tile([128, C], F32, tag="chain", bufs=1,
                                     name="t_ps")
                    for kc2 in range(3):
                        nc.tensor.matmul(
                            t_ps[:],
                            g_sb[:, kc2, c1 * 128:(c1 + 1) * 128],
                            wk_sb[:, kc2, :],
                            start=(kc2 == 0), stop=(kc2 == 2),
                        )
                    nc.scalar.activation(
                        t_sb[:, c1, :], t_ps[:],
                        mybir.ActivationFunctionType.Identity,
                        bias=0.0, scale=1.0)
                st["t"] = t_sb

            def s3():
                g_sb = st["g"]
                uv = []
                for wsb in (wq_sb, wk_sb):
                    r_ps = psum.tile([1, C], F32, tag="chain", bufs=1,
                                     name="r_ps")
                    for kc in range(3):
                        nc.tensor.matmul(
                            r_ps[:], g_sb[:, kc, C:C + 1], wsb[:, kc, :],
                            start=(kc == 0), stop=(kc == 2),
                        )
                    r_sb = sm.tile([1, C], F32R, tag=f"uv{len(uv)}",
                                   name="r_sb")
                    nc.vector.tensor_copy(r_sb[:], r_ps[:])
                    uv.append(r_sb)
                vn = sm.tile([1, C], F32R, tag="vn")
                nc.vector.tensor_scalar_mul(vn[:], bk_row[:], float(N))
                nc.vector.tensor_add(vn[:], vn[:], uv[1][:])
                # stack [u; bq] and [bk; vn] so both rank-1 bias terms fold
                # into ONE K=2 matmul per head; partition-1 rows are not
                # engine-writable, so stage via SBUF->SBUF DMA (gpsimd)
                uq2 = sm.tile([2, C], F32R, tag="uq2")
                kv2 = sm.tile([2, C], F32R, tag="kv2")
                nc.gpsimd.dma_start(uq2[0:1, :], uv[0][:])
                nc.gpsimd.dma_start(uq2[1:2, :], bq_row[:])
                nc.gpsimd.dma_start(kv2[0:1, :], bk_row[:])
                nc.gpsimd.dma_start(kv2[1:2, :], vn[:])
                st["uq2"], st["kv2"] = uq2, kv2

            def s4():
                t_sb, uq2, kv2 = st["t"], st["uq2"], st["kv2"]
                s_ps = psum.tile([48, H, 48], F32, tag="chain", bufs=1,
                                 name="s_ps")
                for h in range(H):
                    hsl = slice(48 * h, 48 * (h + 1))
                    for kc1 in range(3):
                        nc.tensor.matmul(
                            s_ps[:, h, :], wq_sb[:, kc1, hsl],
                            t_sb[:, kc1, hsl],
                            start=(kc1 == 0), stop=False,
                        )
                    nc.tensor.matmul(s_ps[:, h, :], uq2[:, hsl],
                                     kv2[:, hsl], start=False, stop=True)
                p_all = sm.tile([48, H, 48], F32, tag="p_all")
                nc.scalar.activation(
                    p_all[:], s_ps[:], mybir.ActivationFunctionType.Exp,
                    bias=0.0, scale=SCALE)
                zsum = sm.tile([48, H], F32, tag="zsum")
                nc.vector.reduce_sum(zsum[:], p_all[:],
                                     axis=mybir.AxisListType.X)
                zrec = sm.tile([48, H], F32, tag="zrec")
                nc.vector.reciprocal(zrec[:], zsum[:])
                attn = sm.tile([48, H, 48], F32, tag="attn")
                for h in range(H):
                    nc.vector.tensor_scalar_mul(
                        attn[:, h, :], p_all[:, h, :], zrec[:, h:h + 1])
                st["attn"] = attn

            def s5():
                attn = st["attn"]
                # transpose two heads per op: attn[:, h:h+2, :] is [48, 96];
                # its transpose stacks attn_h^T (rows 0:48) over
                # attn_{h+1}^T (rows 48:96)
                tr_ps = psum.tile([96, H // 2, 48], F32, tag="chain", bufs=1,
                                  name="tr_ps")
                for hp in range(H // 2):
                    nc.tensor.transpose(tr_ps[:, hp, :],
                                        attn[:, 2 * hp:2 * hp + 2, :],
                                        id48[:])
                attn_t = sm.tile([96, H // 2, 48], BF16, tag="attn_t")
                nc.scalar.activation(
                    attn_t[:], tr_ps[:],
                    mybir.ActivationFunctionType.Identity,
                    bias=0.0, scale=1.0)
                st["attn_t"] = attn_t

            def s6():
                attn_t = st["attn_t"]
                bd_b = bd[b % 2]
                for h in range(H):
                    hp, e = h // 2, h % 2
                    c0 = 48 * h
                    dj = 0
                    while dj < 48:
                        kc, off = (c0 + dj) // 128, (c0 + dj) % 128
                        cnt = min(48 - dj, 128 - off)
                        nc.gpsimd.dma_start(
                            bd_b[kc][off:off + cnt, c0:c0 + 48],
                            attn_t[48 * e + dj:48 * e + dj + cnt, hp, :])
                        dj += cnt

            return [s1, s2, s3, s4, s5, s6]

        # ---- stage 5 (attn @ v, sparse) + stage 6 (output projection) ----
        def emit_out(b, steps):
            vT_b, _ = state[b]
            at = big1.tile([128, C * NCHUNK], F32R, tag="at", name=f"at{b}")
            atv = at[:].rearrange("p (d h t) -> p h d t", h=H, t=NCHUNK)
            atr = at[:].rearrange("p (r j) -> p r j", j=3)
            bd_b = bd[b % 2]

            for t in range(NCHUNK):
                if t % 3 == 0 and steps:
                    steps.pop(0)()
                at_ps = psum.tile([128, C], F32, tag="g", bufs=5,
                                  name="at_ps")
                for kc in range(3):
                    nc.tensor.matmul(
                        at_ps[:],
                        vT_b[:, kc, t * 128:(t + 1) * 128],
                        bd_b[kc][:],
                        start=(kc == 0), stop=(kc == 2),
                    )
                # 2-op evacuation (DVE+ACT): strided writes are forced by
                # the at layout (stage 6's stride-3 slicing requires it);
                # gpsimd cannot read PSUM, so it takes no share.
                nc.vector.tensor_copy(atv[:, 0:5, :, t], at_ps[:, 0:240])
                nc.scalar.activation(
                    atv[:, 5:8, :, t], at_ps[:, 240:384],
                    mybir.ActivationFunctionType.Identity, bias=0.0, scale=1.0)

            # stage 6: f32r x f32r (bf16 lhsT with f32r rhs is illegal,
            # and bf16 at would make the strided evacuation 2.5x slower)
            for g in range(NG):
                if steps:
                    steps.pop(0)()
                y_sb = yout.tile([128, 4, C], BF16, tag="ysb")
                for j in range(4):
                    rw = 4 * g + j
                    y_ps = psum.tile([128, C], F32, tag="g", bufs=5,
                                     name="y_ps")
                    for jj in range(3):
                        nc.tensor.matmul(
                            y_ps[:],
                            atr[:, rw * 128:(rw + 1) * 128, jj],
                            wp_sb[:, jj, :],
                            start=(jj == 0), stop=(jj == 2),
                        )
                    nc.vector.tensor_add(y_sb[:, j, :], y_ps[:], bp_bc[:])
                nc.sync.dma_start(outp.ap()[b, g], y_sb[:])
            while steps:
                steps.pop(0)()

        emit_vg(0, [])
        emit_vg(1, make_schain(0))
        emit_out(0, make_schain(1))
        emit_out(1, [])

    nc.compile()
    return nc


_CACHE = {}


def prepare_in_maps(x, conv_w, conv_b, wq, bq, wkv, bkv, wp, bp):
    import ml_dtypes

    bf16 = ml_dtypes.bfloat16
    f32 = np.float32
    x = np.ascontiguousarray(x, dtype=f32)

    # fold the 1x1 conv into the projections (host-side weight prep)
    wk_w, wv_w = wkv[:C], wkv[C:]
    bk_b, bv_b = bkv[:C], bkv[C:]
    wqT = np.ascontiguousarray((wq @ conv_w).T, dtype=bf16)
    wkT = np.ascontiguousarray((wk_w @ conv_w).T, dtype=bf16)
    wvT = np.ascontiguousarray((wv_w @ conv_w).T, dtype=bf16)
    wpT = np.ascontiguousarray(wp.T, dtype=f32)
    bq_e = np.ascontiguousarray((bq + wq @ conv_b).reshape(1, C), dtype=f32)
    bk_e = np.ascontiguousarray((bk_b + wk_w @ conv_b).reshape(1, C), dtype=f32)
    bv_e = np.ascontiguousarray(bv_b + wv_w @ conv_b, dtype=f32)
    bp_c = np.ascontiguousarray(bp, dtype=f32)

    xb = x.astype(bf16)
    # window-tiled transposed x: [B, w, p, kc, n]
    xt_t = np.ascontiguousarray(
        xb.transpose(0, 2, 1).reshape(B, 3, 128, NWIN, NW)
        .transpose(0, 3, 2, 1, 4))
    # window-tiled natural x with ones column: [B, w, p, ns, c]
    xn = np.concatenate([xb, np.ones((B, N, 1), dtype=bf16)], axis=2)
    xn_t = np.ascontiguousarray(
        xn.reshape(B, NWIN, 4, 128, C + 1).transpose(0, 1, 3, 2, 4))

    in_maps = []
    for c in range(N_CORES):
        in_maps.append({
            "xt": xt_t[c * BPC:(c + 1) * BPC],
            "xn": xn_t[c * BPC:(c + 1) * BPC],
            "wqT": wqT, "wkT": wkT, "wvT": wvT, "wpT": wpT,
            "bq": bq_e, "bk": bk_e, "bv": bv_e, "bp": bp_c,
        })

    return in_maps


def kernel(x, conv_w, conv_b, wq, bq, wkv, bkv, wp, bp):
    _install_ntff_hook()
    in_maps = prepare_in_maps(x, conv_w, conv_b, wq, bq, wkv, bkv, wp, bp)
    if "nc" not in _CACHE:
        _CACHE["nc"] = build()
    nc = _CACHE["nc"]
    res = run_bass_kernel_spmd(nc, in_maps, core_ids=list(range(N_CORES)))
    # untile: [BPC, NG, 128, 4, C] -> [BPC, N, C]
    outs = []
    for c in range(N_CORES):
        o = np.asarray(res.results[c]["out"])
        outs.append(o.transpose(0, 1, 3, 2, 4).reshape(BPC, N, C))
    return np.concatenate(outs, axis=0).astype(np.float32)
